# revision 1
# baseline (speedup 1.0000x reference)
"""EntityBoundaryPredictor Bass kernel for 8 trn2 NeuronCores.

Reference computation (B=4, E=16, T=1024, H=1024, fp32):
    t   = token_embedding @ Wt + bt                       # [B,T,H]
    e   = entity_embedding @ We + be                      # [B,E,H]
    cls = einsum('beth,h->bet', relu(t[:,None]+e[:,:,None]), Wp) + bp
    cls = where(token_mask, cls, -1e4); p = sigmoid(cls)  # returns (cls, p)

Sharding: data-parallel over (b, token-half): core s -> b = s//2,
tokens [th*512,(th+1)*512) with th = s%2.  Weights replicated.

Per-core device plan (h kept on SBUF partitions throughout):
  - PE: t'(k,t) = Wt^T @ tokenT accumulated over 8 h-chunks into PSUM
        (same for e'); ACT folds the bias in during the PSUM->SBUF copy.
  - DVE/ACT/GpSimd: m = relu(t' + e'_scalar) as one fused per-partition-
        scalar op per (e, h-chunk) [128,512] tile, split across engines.
  - PE: cls partial = Wp^T @ m -- an M=32 matvec per (e, h-chunk), packed
        4-wide into PSUM column groups (partitions 0/32/64/96), with the
        four entity-group accumulators resident in four PSUM banks across
        the whole h loop (h-outer order pipelines behind the projections).
  - ACT/DVE: + bp, select(mask, ., -1e4), sigmoid; DMA the 4 rows out.
"""

import os

import numpy as np

import bass_rust as _bass_rust
import concourse.bacc as bacc
import concourse.mybir as mybir
from concourse.hw_specs import get_activation_tables
from concourse.tile import TileContext
from concourse.bass_utils import run_bass_kernel_spmd

B, E, T, H = 4, 16, 1024, 1024
P = 128
NCORES = 8
TS = T // 2          # tokens per core
HC = H // P          # h-chunks (contraction)
KC = H // P          # k-chunks (projected feature dim; == h of stage 2)
NEG = -10000.0

F32 = mybir.dt.float32
F32R = mybir.dt.float32r
BF16 = mybir.dt.bfloat16
F16 = mybir.dt.float16

# ---- config knobs -----------------------------------------------------------
# in_dt: dtype of the big DRAM inputs + projection matmul operands
#        ('f32' | 'f32r' | 'f16' | 'bf16'). f32r is raw fp32 bits through the
#        PE's single-pass reduced-precision fp32 mode; f16/bf16 mean the host
#        casts while sharding (halves the DMA bytes).
# tp_dt: storage dtype of t'/e-scalar source (elementwise input).
# m_dt:  dtype of the relu'd m tiles and the Wp lhsT.
CFG = {
    "in_dt": os.environ.get("K_IN_DT", "f16"),
    "tp_dt": os.environ.get("K_TP_DT", "f16"),
    "m_dt": os.environ.get("K_M_DT", "f16"),
    # benchmark knob: repeat the computation K times inside one NEFF via a
    # hardware loop (tile tags make reps share SBUF slots -> WAW serialization)
    "reps": int(os.environ.get("K_REPS", "1")),
    # stage bisection for benchmarking: dma | proj | elem | full
    "stage": os.environ.get("K_STAGE", "full"),
    # fraction of relu tiles computed on ACT / GpSimd instead of DVE
    "act_frac": float(os.environ.get("K_ACT_FRAC", "0.3")),
    "gp_frac": float(os.environ.get("K_GP_FRAC", "0.0")),
}

_DT = {"f32": F32, "f32r": F32R, "bf16": BF16, "f16": F16}

LAST_RESULTS = None  # BassKernelResults of the most recent run (for test.py)
_BUILT = None        # (cfg_key, nc)


def build(cfg=None):
    cfg = cfg or CFG
    in_dt = _DT[cfg["in_dt"]]
    tp_dt = _DT[cfg["tp_dt"]]
    m_dt = _DT[cfg["m_dt"]]

    nc = bacc.Bacc("TRN2", target_bir_lowering=False, debug=False)

    # All ACT funcs used here (Identity/Relu/Sigmoid) exist in the
    # sigmoid_and_others set; the default chooser greedily picks
    # exp_and_others for the first two, forcing a ~2.7us table swap per
    # invocation. Blank the other sets (ids preserved) so one load suffices.
    def _one_table_set():
        if not any(
            isinstance(i, mybir.InstActivation)
            for b in nc.main_func.blocks
            for i in b.instructions
        ):
            return
        tables = [
            (n, (f if n == "sigmoid_and_others" else set()))
            for n, f in get_activation_tables(nc.m.arch).items()
        ]
        _bass_rust.insert_act_table_loads(nc, tables)

    nc.insert_act_table_loads = _one_table_set

    tokT = nc.declare_dram_parameter("tokT", [H, TS], in_dt, isOutput=False)
    entT = nc.declare_dram_parameter("entT", [H, E], in_dt, isOutput=False)
    wt = nc.declare_dram_parameter("wt", [H, H], in_dt, isOutput=False)
    we = nc.declare_dram_parameter("we", [H, H], in_dt, isOutput=False)
    smalls = nc.declare_dram_parameter(
        "smalls", [P, TS + 3 * KC + 1], mybir.dt.uint32, isOutput=False
    )

    cls_out = nc.declare_dram_parameter("cls_out", [E, TS], F32, isOutput=True)
    p_out = nc.declare_dram_parameter("p_out", [E, TS], F32, isOutput=True)

    Act = mybir.ActivationFunctionType
    Alu = mybir.AluOpType

    stage = cfg["stage"]
    # engine split pattern for the relu tiles, cycle of 20
    CYC = 20
    gp_n = int(round(cfg["gp_frac"] * CYC))
    act_n = int(round(cfg["act_frac"] * CYC))

    with TileContext(nc) as tc:
        with (
            tc.tile_pool(name="const", bufs=1) as cpool,
            tc.tile_pool(name="mt", bufs=16) as mpool,
            tc.tile_pool(name="fin", bufs=2) as fpool,
            tc.tile_pool(name="psA", bufs=2, space="PSUM") as psA,
            tc.tile_pool(name="psB", bufs=1, space="PSUM") as psB,
            tc.tile_pool(name="psR", bufs=1, space="PSUM") as psR,
        ):
            rep_ctx = tc.For_i(0, cfg["reps"], 1) if cfg["reps"] > 1 else None
            if rep_ctx is not None:
                rep_ctx.__enter__()

            # ---- resident inputs -------------------------------------------
            # few large DMAs, split across the two HWDGE rings; weights are
            # halved by k-columns so the first projections start earlier.
            tok_sb = cpool.tile([P, HC, TS], in_dt, tag="tok")
            HHC = HC // 2
            for tih in range(2):
                hsl = slice(tih * HHC, (tih + 1) * HHC)
                nc.sync.dma_start(
                    out=tok_sb[:, hsl, :],
                    in_=tokT[tih * (H // 2) : (tih + 1) * (H // 2), :].rearrange(
                        "(hc p) t -> p hc t", p=P
                    ),
                )
            ent_sb = cpool.tile([P, HC, E], in_dt, tag="ent")
            nc.sync.dma_start(
                out=ent_sb[:, :, :],
                in_=entT[:, :].rearrange("(hc p) e -> p hc e", p=P),
            )
            wt_sb = cpool.tile([P, HC, H], in_dt, tag="wt")
            we_sb = cpool.tile([P, HC, H], in_dt, tag="we")
            HH = H // 2
            for half in range(2):
                ksl = slice(half * HH, (half + 1) * HH)
                nc.scalar.dma_start(
                    out=wt_sb[:, :, ksl],
                    in_=wt[:, ksl].rearrange("(hc p) k -> p hc k", p=P),
                )
                nc.sync.dma_start(
                    out=we_sb[:, :, ksl],
                    in_=we[:, ksl].rearrange("(hc p) k -> p hc k", p=P),
                )
            NS = TS + 3 * KC + 1
            smalls_sb = cpool.tile([P, NS], mybir.dt.uint32, tag="smalls")
            nc.sync.dma_start(out=smalls_sb[:, :], in_=smalls[:, :])
            mask_sb = smalls_sb[:, 0:TS]
            smalls_f32 = smalls_sb[:, :].bitcast(F32)
            btR_sb = smalls_f32[:, TS : TS + KC]
            beR_sb = smalls_f32[:, TS + KC : TS + 2 * KC]
            wpR_sb = smalls_f32[:, TS + 2 * KC : TS + 3 * KC]
            bpR_sb = smalls_f32[:, TS + 3 * KC : TS + 3 * KC + 1]

            # output staging tiles, preset on the otherwise-idle GpSimd:
            # cls rows default to NEG, p rows default to 0 (masked values).
            clsM_t = []
            pZ_t = []
            for eg in range(E // 4):
                cm = cpool.tile([P, TS], F32, tag=f"clsM{eg}", name=f"clsM{eg}")
                nc.gpsimd.memset(cm[:, :], NEG)
                clsM_t.append(cm)
                pz = cpool.tile([P, TS], F32, tag=f"pZ{eg}", name=f"pZ{eg}")
                nc.gpsimd.memset(pz[:, :], 0.0)
                pZ_t.append(pz)

            # Wp in the reduce-matmul dtype, replicated to 32 lhsT columns per
            # h-chunk so the M=32 matvec initializes a full PSUM column group.
            wp_sb = cpool.tile([P, KC, 32], m_dt, tag="wp")
            for kc in range(KC):
                nc.vector.tensor_copy(
                    out=wp_sb[:, kc, :],
                    in_=wpR_sb[:, kc : kc + 1].broadcast_to([P, 32]),
                )

            # ---- projections -----------------------------------------------
            tp_sb = cpool.tile([P, KC, TS], tp_dt, tag="tp")   # t' [k, t]
            ep_sb = cpool.tile([P, KC, E], F32, tag="ep")      # e' [k, e]
            if stage != "dma":
                for kc in range(KC):
                    ps = psA.tile([P, TS], F32, tag="ps_proj")
                    for hc in range(HC):
                        nc.tensor.matmul(
                            ps[:, :],
                            lhsT=wt_sb[:, hc, kc * P : (kc + 1) * P],
                            rhs=tok_sb[:, hc, :],
                            start=(hc == 0),
                            stop=(hc == HC - 1),
                        )
                    nc.scalar.activation(
                        tp_sb[:, kc, :], ps[:, :], Act.Identity,
                        bias=btR_sb[:, kc : kc + 1],
                    )
                    eps = psB.tile([P, E], F32, tag="ps_eproj")
                    for hc in range(HC):
                        nc.tensor.matmul(
                            eps[:, :],
                            lhsT=we_sb[:, hc, kc * P : (kc + 1) * P],
                            rhs=ent_sb[:, hc, :],
                            start=(hc == 0),
                            stop=(hc == HC - 1),
                        )
                    nc.scalar.activation(
                        ep_sb[:, kc, :], eps[:, :], Act.Identity,
                        bias=beR_sb[:, kc : kc + 1],
                    )

            # ---- relu(t'+e') + weighted reduction over h (h-outer) ---------
            if stage in ("elem", "full"):
                rps = [psR.tile([P, TS], F32, tag=f"ps_red{eg}",
                                name=f"rps{eg}")
                       for eg in range(E // 4)]
                g_tile = 0
                for hc in range(HC):
                    for e in range(E):
                        eg, j = divmod(e, 4)
                        m = mpool.tile([P, TS], m_dt, tag="m")
                        lane = g_tile % CYC
                        g_tile += 1
                        if lane < gp_n:
                            nc.gpsimd.tensor_scalar(
                                out=m[:, :],
                                in0=tp_sb[:, hc, :],
                                scalar1=ep_sb[:, hc, e : e + 1],
                                scalar2=0.0,
                                op0=Alu.add,
                                op1=Alu.max,
                            )
                        elif lane < gp_n + act_n:
                            nc.scalar.activation(
                                m[:, :], tp_sb[:, hc, :], Act.Relu,
                                bias=ep_sb[:, hc, e : e + 1],
                            )
                        else:
                            nc.vector.tensor_scalar(
                                out=m[:, :],
                                in0=tp_sb[:, hc, :],
                                scalar1=ep_sb[:, hc, e : e + 1],
                                scalar2=0.0,
                                op0=Alu.add,
                                op1=Alu.max,
                            )
                        if stage == "full":
                            nc.tensor.matmul(
                                rps[eg][32 * j : 32 * j + 32, :],
                                lhsT=wp_sb[:, hc, :],
                                rhs=m[:, :],
                                start=(hc == 0),
                                stop=(hc == HC - 1),
                                tile_position=(0, 32 * j),
                                # the 4 column groups interleave accumulation
                                # in one bank on disjoint partition ranges;
                                # the group tracker is partition-unaware.
                                skip_group_check=True,
                            )

                # ---- finalize: +bp, mask, sigmoid, store -------------------
                if stage == "full":
                    for eg in range(E // 4):
                        clsT = fpool.tile([P, TS], F32, tag="clsT")
                        nc.scalar.activation(
                            clsT[:, :], rps[eg][:, :], Act.Identity,
                            bias=bpR_sb[:, 0:1],
                        )
                        pS = fpool.tile([P, TS], F32, tag="pS")
                        nc.scalar.activation(
                            pS[:, :], rps[eg][:, :], Act.Sigmoid,
                            bias=bpR_sb[:, 0:1],
                        )
                        nc.vector.copy_predicated(
                            clsM_t[eg][:, :], mask_sb[:, :], clsT[:, :]
                        )
                        nc.vector.copy_predicated(
                            pZ_t[eg][:, :], mask_sb[:, :], pS[:, :]
                        )
                        cls_rows = clsM_t[eg][:, :].rearrange(
                            "(a b) t -> a b t", b=32)[:, 0, :]
                        p_rows = pZ_t[eg][:, :].rearrange(
                            "(a b) t -> a b t", b=32)[:, 0, :]
                        nc.sync.dma_start(
                            out=cls_out[eg * 4 : eg * 4 + 4, :], in_=cls_rows
                        )
                        nc.sync.dma_start(
                            out=p_out[eg * 4 : eg * 4 + 4, :], in_=p_rows
                        )

            if rep_ctx is not None:
                rep_ctx.__exit__(None, None, None)

    nc.compile()
    return nc


def _np_dt(name):
    import ml_dtypes

    return {"f32": np.float32, "f32r": np.float32, "bf16": ml_dtypes.bfloat16,
            "f16": np.float16}[name]


def shard_inputs(token_embedding, entity_embedding, token_mask, Wt, bt, We, be,
                 Wp, bp, cfg=None):
    cfg = cfg or CFG
    ind = _np_dt(cfg["in_dt"])
    f32 = np.float32

    wt_s = np.ascontiguousarray(Wt.astype(ind, copy=False))
    we_s = np.ascontiguousarray(We.astype(ind, copy=False))
    btR = np.ascontiguousarray(bt.astype(f32).reshape(KC, P).T)
    beR = np.ascontiguousarray(be.astype(f32).reshape(KC, P).T)
    wpR = np.ascontiguousarray(Wp.astype(f32).reshape(KC, P).T)
    bpR = np.broadcast_to(bp.astype(f32).reshape(1, 1), (P, 1))

    consts = np.concatenate(
        [btR.view(np.uint32), beR.view(np.uint32), wpR.view(np.uint32),
         np.ascontiguousarray(bpR).view(np.uint32)], axis=1,
    )

    in_maps = []
    for s in range(NCORES):
        b, th = divmod(s, 2)
        tsl = slice(th * TS, (th + 1) * TS)
        tokT = np.ascontiguousarray(
            token_embedding[b, tsl, :].T.astype(ind, copy=False))
        entT = np.ascontiguousarray(entity_embedding[b].T.astype(ind, copy=False))
        maskR = np.broadcast_to(
            token_mask[b, tsl].astype(np.uint32)[None, :], (P, TS))
        smalls = np.ascontiguousarray(np.concatenate([maskR, consts], axis=1))
        in_maps.append({
            "tokT": tokT, "entT": entT, "wt": wt_s, "we": we_s, "smalls": smalls,
        })
    return in_maps


def kernel(token_embedding, entity_embedding, token_mask, Wt, bt, We, be, Wp, bp):
    global LAST_RESULTS, _BUILT
    cfg_key = tuple(sorted(CFG.items()))
    if _BUILT is None or _BUILT[0] != cfg_key:
        _BUILT = (cfg_key, build(CFG))
    nc = _BUILT[1]

    in_maps = shard_inputs(token_embedding, entity_embedding, token_mask,
                           Wt, bt, We, be, Wp, bp)
    trace = os.environ.get("K_TRACE", "0") == "1"
    res = run_bass_kernel_spmd(nc, in_maps, core_ids=list(range(NCORES)),
                               trace=trace)
    LAST_RESULTS = res

    cls = np.empty((B, E, T), np.float32)
    p = np.empty((B, E, T), np.float32)
    for s in range(NCORES):
        b, th = divmod(s, 2)
        tsl = slice(th * TS, (th + 1) * TS)
        cls[b, :, tsl] = res.results[s]["cls_out"]
        p[b, :, tsl] = res.results[s]["p_out"]
    return cls, p

